# revision 12
# baseline (speedup 1.0000x reference)
"""Brute-force KNN retrieval (B=512 queries, N=500000 candidates, D=128, top-K)
on 8 Trainium2 NeuronCores.

Strategy: candidates sharded along N across the 8 cores, queries replicated.
Per core, per (chunk c, query-tile m) PSUM tile of 2048 fp32 scores:

  ACT:  a_c = f16(ps_c[1024:2048])            (hi-half copy to SBUF)
  DVE:  s1_c = fp8(max(ps_c[0:1024], a_{c-1}))  (folds the PREVIOUS chunk's hi
        copy: ACT and DVE run concurrently on a tile, PSUM frees after
        max(ACT, DVE), and every PSUM element is touched exactly once by
        exactly one engine)
  plus a per-m flush of a_30 after the chunk loop.

s1 holds 1024 fp8e4m3 pairwise maxima per tile, slot j covering candidates
{c*2048+j, (c-1)*2048+1024+j}. They are DMA'd out as [B, 32*1024] fp8 per
core. A slot's max is >= both members' scores and fp8 rounding is monotone,
so the <=100 slots holding a row's true top-100 always rank within the top-R
fp8 values for R >> (#candidates within ~2 fp8 ulp of the rank-100 score,
~700 here); we use R=1024. The host expands the top-R slots to <=2R candidate
ids, rescores them exactly in fp32, and emits the exact global top-K
(ties -> lower index, matching lax.top_k).
"""

import sys

for _p in ("/opt/trn_rl_repo",):
    if _p not in sys.path:
        sys.path.insert(0, _p)

import numpy as np

B, N, D = 512, 500000, 128
N_CORES = 8
SHARD = N // N_CORES          # 62500 candidates per core
PCHUNK = 2048                 # PSUM tile width (4 banks)
NCHUNK = -(-SHARD // PCHUNK)  # 31
PADN = PCHUNK * NCHUNK        # 63488 (padded shard width)
NSUB = PCHUNK // 512          # 4 matmuls per PSUM tile
MTILES = B // 128             # 4 query tiles
SPC = PCHUNK // 2             # 1024 survivors (pair maxima) per tile
SURV = (NCHUNK + 1) * SPC     # 32768 survivor slots per (row, core): 31 + flush
FULLC = (7, 23)               # chunks drained entirely by ACT (f16 survivors)
RBLOCKS = 1024                # host rescores this many slots per row
NEG = -60000.0                # -inf stand-in for the f16 bootstrap tile

_NC_CACHE = {}


def _build_nc():
    import concourse.bacc as bacc
    import concourse.tile as tile
    import concourse.mybir as mybir

    f32 = mybir.dt.float32
    f16 = mybir.dt.float16
    fp8 = mybir.dt.float8e4
    bf16 = mybir.dt.bfloat16
    mx = mybir.AluOpType.max

    nc = bacc.Bacc(
        "TRN2", target_bir_lowering=False, debug=False, num_devices=N_CORES
    )
    qT = nc.dram_tensor("qT", [D, B], bf16, kind="ExternalInput")
    cT = nc.dram_tensor("cT", [D, PADN], bf16, kind="ExternalInput")
    surv = nc.dram_tensor("surv", [B, SURV], fp8, kind="ExternalOutput")
    surv2 = nc.dram_tensor("surv2", [B, len(FULLC) * SPC], f16,
                           kind="ExternalOutput")

    with tile.TileContext(nc) as tc:
        with (
            tc.tile_pool(name="q", bufs=1) as qp,
            tc.tile_pool(name="c", bufs=6) as cp,
            tc.tile_pool(name="pa", bufs=2, space="PSUM") as pap,
            tc.tile_pool(name="pb", bufs=2, space="PSUM") as pbp,
            tc.tile_pool(name="ph", bufs=2, space="PSUM") as php,
            tc.tile_pool(name="a", bufs=14) as ap_,
            tc.tile_pool(name="s1", bufs=10) as s1p,
        ):
            qt = qp.tile([128, B], bf16)
            nc.sync.dma_start(qt[:], qT.ap())

            ainit = qp.tile([128, 1024], f16, name="ainit")
            nc.vector.memset(ainit[:], NEG)

            aprev = {m: ainit for m in range(MTILES)}

            for c in range(NCHUNK):
                ct = cp.tile([128, PCHUNK], bf16, name=f"ct{c}", tag="ct")
                nc.sync.dma_start(ct[:], cT.ap()[:, c * PCHUNK:(c + 1) * PCHUNK])
                for m in range(MTILES):
                    # lo half as two 1-bank tiles (DVE-read, fast release),
                    # hi half as one 2-bank tile (ACT-read); separate sems
                    # let each fold start right after its own matmul
                    pa = pap.tile([128, 512], f32, name=f"pa{c}_{m}", tag="pa")
                    pb = pbp.tile([128, 512], f32, name=f"pb{c}_{m}", tag="pb")
                    ph = php.tile([128, 1024], f32, name=f"ph{c}_{m}", tag="ph")
                    qs = qt[:, m * 128:(m + 1) * 128]
                    nc.tensor.matmul(pa[:], qs, ct[:, 0:512],
                                     start=True, stop=True)
                    nc.tensor.matmul(pb[:], qs, ct[:, 512:1024],
                                     start=True, stop=True)
                    nc.tensor.matmul(ph[:, 0:512], qs, ct[:, 1024:1536],
                                     start=True, stop=True)
                    nc.tensor.matmul(ph[:, 512:1024], qs, ct[:, 1536:2048],
                                     start=True, stop=True)
                    a = ap_.tile([128, 1024], f16, name=f"a{c}_{m}", tag="a")
                    nc.scalar.activation(
                        a[:], ph[:],
                        mybir.ActivationFunctionType.Copy,
                        bias=0.0, scale=1.0,
                    )
                    if c in FULLC:
                        # ACT-full tile: ACT also drains the lo half; DVE
                        # does only the cheap f16 fold (2x mode, f16 out)
                        alo = ap_.tile([128, 1024], f16,
                                       name=f"alo{c}_{m}", tag="alo")
                        nc.scalar.activation(
                            alo[:, 0:512], pa[:],
                            mybir.ActivationFunctionType.Copy,
                            bias=0.0, scale=1.0,
                        )
                        nc.scalar.activation(
                            alo[:, 512:1024], pb[:],
                            mybir.ActivationFunctionType.Copy,
                            bias=0.0, scale=1.0,
                        )
                        s1f = s1p.tile([128, SPC], f16,
                                       name=f"s1f_{c}_{m}", tag="s1w")
                        nc.vector.tensor_tensor(
                            s1f[:], alo[:], aprev[m][:], op=mx
                        )
                        fi = FULLC.index(c)
                        nc.sync.dma_start(
                            surv2.ap()[m * 128:(m + 1) * 128,
                                       fi * SPC:(fi + 1) * SPC],
                            s1f[:],
                        )
                    else:
                        s1 = s1p.tile([128, SPC], fp8,
                                      name=f"s1_{c}_{m}", tag="s1")
                        nc.vector.tensor_tensor(
                            s1[:, 0:512], pa[:], aprev[m][:, 0:512], op=mx
                        )
                        nc.vector.tensor_tensor(
                            s1[:, 512:1024], pb[:], aprev[m][:, 512:1024], op=mx
                        )
                        nc.sync.dma_start(
                            surv.ap()[m * 128:(m + 1) * 128,
                                      c * SPC:(c + 1) * SPC],
                            s1[:],
                        )
                    aprev[m] = a

            # flush the last chunk's unfolded hi copies
            for m in range(MTILES):
                s1 = s1p.tile([128, SPC], fp8, name=f"s1f_{m}", tag="s1")
                nc.scalar.activation(
                    s1[:], aprev[m][:], mybir.ActivationFunctionType.Copy,
                    bias=0.0, scale=1.0,
                )
                nc.sync.dma_start(
                    surv.ap()[m * 128:(m + 1) * 128, NCHUNK * SPC:(NCHUNK + 1) * SPC],
                    s1[:],
                )

    nc.compile()
    return nc


def _get_nc():
    if "nc" not in _NC_CACHE:
        _NC_CACHE["nc"] = _build_nc()
    return _NC_CACHE["nc"]


def _make_in_maps(queries, candidates):
    import ml_dtypes

    bf = ml_dtypes.bfloat16
    q = np.asarray(queries, dtype=np.float32)
    cand = np.asarray(candidates, dtype=np.float32)
    qTh = np.ascontiguousarray(q.T.astype(bf))  # [D, B] bf16
    in_maps = []
    for i in range(N_CORES):
        cTi = np.zeros((D, PADN), dtype=bf)
        cTi[:, :SHARD] = cand[i * SHARD:(i + 1) * SHARD].T.astype(bf)
        in_maps.append({"qT": qTh, "cT": cTi})
    return in_maps


def _run_device(in_maps, trace=False):
    from concourse import bass_utils

    nc = _get_nc()
    return bass_utils.run_bass_kernel_spmd(
        nc, in_maps, core_ids=list(range(N_CORES)), trace=trace
    )


_BLOCKS_CACHE = {}


def _block_members():
    """[SURV, 2] local candidate positions per survivor slot (same for all m).

    -1 marks an invalid member. Positions are within the core's padded
    shard [0, PADN); entries >= SHARD are padding, masked by the caller.
    """
    if "b" in _BLOCKS_CACHE:
        return _BLOCKS_CACHE["b"]
    mem = np.full((SURV, 2), -1, dtype=np.int64)
    j = np.arange(SPC)
    for c in range(NCHUNK):
        k = c * SPC + j
        mem[k, 0] = c * PCHUNK + j                     # lo half of chunk c
        if c > 0:
            mem[k, 1] = (c - 1) * PCHUNK + 1024 + j    # hi half of chunk c-1
    k = NCHUNK * SPC + j                               # flush: hi of chunk 30
    mem[k, 0] = (NCHUNK - 1) * PCHUNK + 1024 + j
    _BLOCKS_CACHE["b"] = mem
    return mem


def _merge(results, queries, candidates, identifiers, num_candidates):
    K = int(num_candidates)
    q = np.asarray(queries, dtype=np.float32)
    cand = np.asarray(candidates, dtype=np.float32)
    mem = _block_members()                                       # [SURV, 2]

    per_core = []
    for i in range(N_CORES):
        v = np.asarray(results[i]["surv"], dtype=np.float32)
        v2 = np.asarray(results[i]["surv2"], dtype=np.float32)
        for fi, c in enumerate(FULLC):
            v[:, c * SPC:(c + 1) * SPC] = v2[:, fi * SPC:(fi + 1) * SPC]
        per_core.append(v)
    vals = np.concatenate(per_core, axis=1)  # [B, 8*SURV]
    nblk = vals.shape[1]
    r = min(RBLOCKS, nblk)
    part = np.argpartition(vals, nblk - r, axis=1)[:, -r:]       # [B, r]
    core_of = part // SURV
    k_of = part % SURV
    pos = mem[k_of]                                              # [B, r, 2]
    validity = (pos >= 0) & (pos < SHARD)
    gids3 = core_of[:, :, None] * SHARD + np.clip(pos, 0, SHARD - 1)
    gids = gids3.reshape(B, -1)                                  # [B, 2r]
    valid = validity.reshape(B, -1)

    out_vals = np.empty((B, K), dtype=np.float32)
    out_idx = np.empty((B, K), dtype=np.int64)
    bs = 128
    for b0 in range(0, B, bs):
        b1 = min(b0 + bs, B)
        g = gids[b0:b1]                                          # [bb, 2r]
        csel = cand[g]                                           # [bb, 2r, D]
        vsel = np.einsum("bjd,bd->bj", csel, q[b0:b1], dtype=np.float32)
        vsel = np.where(valid[b0:b1], vsel, -np.inf)
        # drop duplicate ids within a row (invalid-clip can create dupes)
        order_g = np.argsort(g, axis=1, kind="stable")
        g_sorted = np.take_along_axis(g, order_g, axis=1)
        dup = np.zeros_like(g_sorted, dtype=bool)
        dup[:, 1:] = g_sorted[:, 1:] == g_sorted[:, :-1]
        dup_unsorted = np.zeros_like(dup)
        np.put_along_axis(dup_unsorted, order_g, dup, axis=1)
        vsel = np.where(dup_unsorted, -np.inf, vsel)
        order = np.lexsort((g, -vsel), axis=-1)[:, :K]
        out_vals[b0:b1] = np.take_along_axis(vsel, order, axis=1)
        out_idx[b0:b1] = np.take_along_axis(g, order, axis=1)

    ids = np.asarray(identifiers)
    out_ids = np.take(ids, out_idx, axis=0)
    return out_vals, out_ids


def kernel(queries, candidates, identifiers, num_candidates):
    in_maps = _make_in_maps(queries, candidates)
    res = _run_device(in_maps, trace=False)
    return _merge(res.results, queries, candidates, identifiers, num_candidates)


# revision 14
# speedup vs baseline: 1.0001x; 1.0001x over previous
"""Brute-force KNN retrieval (B=512 queries, N=500000 candidates, D=128, top-K)
on 8 Trainium2 NeuronCores.

Strategy: candidates sharded along N across the 8 cores, queries replicated.
Per core, per (chunk c, query-tile m) PSUM tile of 2048 fp32 scores:

  ACT:  a_c = f16(ps_c[1024:2048])            (hi-half copy to SBUF)
  DVE:  s1_c = fp8(max(ps_c[0:1024], a_{c-1}))  (folds the PREVIOUS chunk's hi
        copy: ACT and DVE run concurrently on a tile, PSUM frees after
        max(ACT, DVE), and every PSUM element is touched exactly once by
        exactly one engine)
  plus a per-m flush of a_30 after the chunk loop.

s1 holds 1024 fp8e4m3 pairwise maxima per tile, slot j covering candidates
{c*2048+j, (c-1)*2048+1024+j}. They are DMA'd out as [B, 32*1024] fp8 per
core. A slot's max is >= both members' scores and fp8 rounding is monotone,
so the <=100 slots holding a row's true top-100 always rank within the top-R
fp8 values for R >> (#candidates within ~2 fp8 ulp of the rank-100 score,
~700 here); we use R=1024. The host expands the top-R slots to <=2R candidate
ids, rescores them exactly in fp32, and emits the exact global top-K
(ties -> lower index, matching lax.top_k).
"""

import sys

for _p in ("/opt/trn_rl_repo",):
    if _p not in sys.path:
        sys.path.insert(0, _p)

import numpy as np

B, N, D = 512, 500000, 128
N_CORES = 8
SHARD = N // N_CORES          # 62500 candidates per core
PCHUNK = 2048                 # PSUM tile width (4 banks)
NCHUNK = -(-SHARD // PCHUNK)  # 31
PADN = PCHUNK * NCHUNK        # 63488 (padded shard width)
NSUB = PCHUNK // 512          # 4 matmuls per PSUM tile
MTILES = B // 128             # 4 query tiles
SPC = PCHUNK // 2             # 1024 survivors (pair maxima) per tile
SURV = (NCHUNK + 1) * SPC     # 32768 survivor slots per (row, core): 31 + flush
RBLOCKS = 1024                # host rescores this many slots per row
NEG = -60000.0                # -inf stand-in for the f16 bootstrap tile

_NC_CACHE = {}


def _build_nc():
    import concourse.bacc as bacc
    import concourse.tile as tile
    import concourse.mybir as mybir

    f32 = mybir.dt.float32
    f16 = mybir.dt.float16
    fp8 = mybir.dt.float8e4
    bf16 = mybir.dt.bfloat16
    mx = mybir.AluOpType.max

    nc = bacc.Bacc(
        "TRN2", target_bir_lowering=False, debug=False, num_devices=N_CORES
    )
    qT = nc.dram_tensor("qT", [D, B], bf16, kind="ExternalInput")
    cT = nc.dram_tensor("cT", [D, PADN], bf16, kind="ExternalInput")
    surv = nc.dram_tensor("surv", [B, SURV], fp8, kind="ExternalOutput")

    with tile.TileContext(nc) as tc:
        with (
            tc.tile_pool(name="q", bufs=1) as qp,
            tc.tile_pool(name="c", bufs=6) as cp,
            tc.tile_pool(name="pa", bufs=2, space="PSUM") as pap,
            tc.tile_pool(name="pb", bufs=2, space="PSUM") as pbp,
            tc.tile_pool(name="ph", bufs=2, space="PSUM") as php,
            tc.tile_pool(name="a", bufs=14) as ap_,
            tc.tile_pool(name="s1", bufs=10) as s1p,
        ):
            qt = qp.tile([128, B], bf16)
            nc.sync.dma_start(qt[:], qT.ap())

            ainit = qp.tile([128, 1024], f16, name="ainit")
            nc.vector.memset(ainit[:], NEG)

            aprev = {m: ainit for m in range(MTILES)}

            for c in range(NCHUNK):
                ct = cp.tile([128, PCHUNK], bf16, name=f"ct{c}", tag="ct")
                nc.sync.dma_start(ct[:], cT.ap()[:, c * PCHUNK:(c + 1) * PCHUNK])
                for m in range(MTILES):
                    # lo half as two 1-bank tiles (DVE-read, fast release),
                    # hi half as one 2-bank tile (ACT-read); separate sems
                    # let each fold start right after its own matmul
                    pa = pap.tile([128, 512], f32, name=f"pa{c}_{m}", tag="pa")
                    pb = pbp.tile([128, 512], f32, name=f"pb{c}_{m}", tag="pb")
                    ph = php.tile([128, 1024], f32, name=f"ph{c}_{m}", tag="ph")
                    qs = qt[:, m * 128:(m + 1) * 128]
                    nc.tensor.matmul(pa[:], qs, ct[:, 0:512],
                                     start=True, stop=True)
                    nc.tensor.matmul(pb[:], qs, ct[:, 512:1024],
                                     start=True, stop=True)
                    nc.tensor.matmul(ph[:, 0:512], qs, ct[:, 1024:1536],
                                     start=True, stop=True)
                    nc.tensor.matmul(ph[:, 512:1024], qs, ct[:, 1536:2048],
                                     start=True, stop=True)
                    a = ap_.tile([128, 1024], f16, name=f"a{c}_{m}", tag="a")
                    nc.scalar.activation(
                        a[:], ph[:],
                        mybir.ActivationFunctionType.Copy,
                        bias=0.0, scale=1.0,
                    )
                    s1 = s1p.tile([128, SPC], fp8, name=f"s1_{c}_{m}", tag="s1")
                    nc.vector.tensor_tensor(
                        s1[:, 0:512], pa[:], aprev[m][:, 0:512], op=mx
                    )
                    nc.vector.tensor_tensor(
                        s1[:, 512:1024], pb[:], aprev[m][:, 512:1024], op=mx
                    )
                    aprev[m] = a
                    nc.sync.dma_start(
                        surv.ap()[m * 128:(m + 1) * 128, c * SPC:(c + 1) * SPC],
                        s1[:],
                    )

            # flush the last chunk's unfolded hi copies
            for m in range(MTILES):
                s1 = s1p.tile([128, SPC], fp8, name=f"s1f_{m}", tag="s1")
                nc.scalar.activation(
                    s1[:], aprev[m][:], mybir.ActivationFunctionType.Copy,
                    bias=0.0, scale=1.0,
                )
                nc.sync.dma_start(
                    surv.ap()[m * 128:(m + 1) * 128, NCHUNK * SPC:(NCHUNK + 1) * SPC],
                    s1[:],
                )

    nc.compile()
    return nc


def _get_nc():
    if "nc" not in _NC_CACHE:
        _NC_CACHE["nc"] = _build_nc()
    return _NC_CACHE["nc"]


def _make_in_maps(queries, candidates):
    import ml_dtypes

    bf = ml_dtypes.bfloat16
    q = np.asarray(queries, dtype=np.float32)
    cand = np.asarray(candidates, dtype=np.float32)
    qTh = np.ascontiguousarray(q.T.astype(bf))  # [D, B] bf16
    in_maps = []
    for i in range(N_CORES):
        cTi = np.zeros((D, PADN), dtype=bf)
        cTi[:, :SHARD] = cand[i * SHARD:(i + 1) * SHARD].T.astype(bf)
        in_maps.append({"qT": qTh, "cT": cTi})
    return in_maps


def _run_device(in_maps, trace=False):
    from concourse import bass_utils

    nc = _get_nc()
    return bass_utils.run_bass_kernel_spmd(
        nc, in_maps, core_ids=list(range(N_CORES)), trace=trace
    )


_BLOCKS_CACHE = {}


def _block_members():
    """[SURV, 2] local candidate positions per survivor slot (same for all m).

    -1 marks an invalid member. Positions are within the core's padded
    shard [0, PADN); entries >= SHARD are padding, masked by the caller.
    """
    if "b" in _BLOCKS_CACHE:
        return _BLOCKS_CACHE["b"]
    mem = np.full((SURV, 2), -1, dtype=np.int64)
    j = np.arange(SPC)
    for c in range(NCHUNK):
        k = c * SPC + j
        mem[k, 0] = c * PCHUNK + j                     # lo half of chunk c
        if c > 0:
            mem[k, 1] = (c - 1) * PCHUNK + 1024 + j    # hi half of chunk c-1
    k = NCHUNK * SPC + j                               # flush: hi of chunk 30
    mem[k, 0] = (NCHUNK - 1) * PCHUNK + 1024 + j
    _BLOCKS_CACHE["b"] = mem
    return mem


def _merge(results, queries, candidates, identifiers, num_candidates):
    K = int(num_candidates)
    q = np.asarray(queries, dtype=np.float32)
    cand = np.asarray(candidates, dtype=np.float32)
    mem = _block_members()                                       # [SURV, 2]

    vals = np.concatenate(
        [np.asarray(results[i]["surv"], dtype=np.float32) for i in range(N_CORES)],
        axis=1,
    )  # [B, 8*SURV]
    nblk = vals.shape[1]
    r = min(RBLOCKS, nblk)
    # two-level top-r: a slot in the global top-r always lives in a group
    # whose max is within the top-r group maxima (max >= member), so the
    # coarse prefilter is lossless
    G = 16
    ngr = nblk // G
    gmax = vals.reshape(B, ngr, G).max(axis=2)                   # [B, ngr]
    tg = min(r, ngr)
    gpart = np.argpartition(gmax, ngr - tg, axis=1)[:, -tg:]     # [B, tg]
    slots = (gpart[:, :, None] * G
             + np.arange(G)[None, None, :]).reshape(B, -1)       # [B, tg*G]
    sv = np.take_along_axis(vals, slots, axis=1)
    sp_ = np.argpartition(sv, sv.shape[1] - r, axis=1)[:, -r:]
    part = np.take_along_axis(slots, sp_, axis=1)                # [B, r]
    core_of = part // SURV
    k_of = part % SURV
    pos = mem[k_of]                                              # [B, r, 2]
    validity = (pos >= 0) & (pos < SHARD)
    gids3 = core_of[:, :, None] * SHARD + np.clip(pos, 0, SHARD - 1)
    gids = gids3.reshape(B, -1)                                  # [B, 2r]
    valid = validity.reshape(B, -1)

    out_vals = np.empty((B, K), dtype=np.float32)
    out_idx = np.empty((B, K), dtype=np.int64)
    bs = 128
    for b0 in range(0, B, bs):
        b1 = min(b0 + bs, B)
        g = gids[b0:b1]                                          # [bb, 2r]
        csel = cand[g]                                           # [bb, 2r, D]
        vsel = np.einsum("bjd,bd->bj", csel, q[b0:b1], dtype=np.float32)
        vsel = np.where(valid[b0:b1], vsel, -np.inf)
        # drop duplicate ids within a row (invalid-clip can create dupes)
        order_g = np.argsort(g, axis=1, kind="stable")
        g_sorted = np.take_along_axis(g, order_g, axis=1)
        dup = np.zeros_like(g_sorted, dtype=bool)
        dup[:, 1:] = g_sorted[:, 1:] == g_sorted[:, :-1]
        dup_unsorted = np.zeros_like(dup)
        np.put_along_axis(dup_unsorted, order_g, dup, axis=1)
        vsel = np.where(dup_unsorted, -np.inf, vsel)
        order = np.lexsort((g, -vsel), axis=-1)[:, :K]
        out_vals[b0:b1] = np.take_along_axis(vsel, order, axis=1)
        out_idx[b0:b1] = np.take_along_axis(g, order, axis=1)

    ids = np.asarray(identifiers)
    out_ids = np.take(ids, out_idx, axis=0)
    return out_vals, out_ids


def kernel(queries, candidates, identifiers, num_candidates):
    in_maps = _make_in_maps(queries, candidates)
    res = _run_device(in_maps, trace=False)
    return _merge(res.results, queries, candidates, identifiers, num_candidates)
